# revision 21
# baseline (speedup 1.0000x reference)
"""Trainium2 Bass kernel for nn_CapRNNModelHelper (bi-GRU + capsule routing).

Sharding: data-parallel over batch across 8 cores (16 batch rows per core).
Everything else (embedding table, GRU weights, capsule weights) replicated.

Per-core pipeline (fp16 operands, f32 accumulation):
  1. 32 indirect-DMA gathers of fp16 embedding rows (padded to 384 dims)
  2. 32 XBAR DMA-transposes -> eT [dim(3x128 chunks), token] fp16 (no PE)
  3. x_proj matmuls (fp16), biases folded, z negated so sigmoid gives w=1-z
  4. chunked-parallel bidirectional GRU scan: 16 chunks/dir x (10 warmup +
     16 real) steps; h state lives directly in the hbf history buffer
     (fp16); per step: PSUM ident+gate matmuls, sigmoid/tanh on ACT,
     fp16 2x-mode elementwise on DVE (+1 op on Pool)
  5. capsule matmul (fp16, dc-major columns) -> u_hat [tok, 160] fp16
  6. 5-iter dynamic routing: fp16 selB matmuls for sequence reduction,
     exp/ln-only activations (no act-table thrash), tree du-reduction
  7. final linear -> out [16, 2]
"""

import numpy as np
from contextlib import ExitStack

import concourse.bass as bass
import concourse.tile as tile
from concourse import mybir
from concourse.bass import IndirectOffsetOnAxis
from concourse.bass_utils import run_bass_kernel_spmd
from concourse.tile_rust import add_dep_helper

F32 = mybir.dt.float32
F16 = mybir.dt.float16
I32 = mybir.dt.int32
AF = mybir.ActivationFunctionType
OP = mybir.AluOpType
AX = mybir.AxisListType

VOCAB, D_W, H, S, B = 50000, 300, 128, 256, 128
DWP = 384                  # padded embedding width (3 k-chunks of 128)
NUM_CAP, DIM_CAP, ROUTINGS, EPS = 10, 16, 5, 1e-7
NCORES = 8
BL = B // NCORES           # 16 batch rows per core
NTOK = S * BL              # 4096 tokens per core
NGRP = NTOK // 128         # 32 gather groups of 128 tokens
G3 = 3 * H                 # 384

RZW = 4 * BL               # 64   per-step rz width [rf zf rb zb]
NW = 2 * BL                # 32   per-step n width [nf nb]
PCH = 16                   # parallel chunks per direction in the scan
CCH = S // PCH             # 16 real steps per chunk
WU = 8                     # warmup steps (approx. state rebuild) per chunk
EXT = S + 2 * WU           # padded xp timeline
PB = PCH * BL              # 256  per-step per-dir instruction width
HB0 = (S + 1) * BL         # backward base inside token-major hbf
# hbf_s layout: slot-major so scan writes are contiguous [1, PB]:
#   h_f(s) at (s % CCH + 1)*PB + (s // CCH)*BL + bl   (slots 1..CCH)
#   h_b(s) at HB2 + (s % CCH)*PB + (s // CCH)*BL + bl (slots 0..CCH-1)
HB2 = (CCH + 1) * PB       # backward base inside slot-major hbf_s
# block index for (dir d, gate g): rz blocks 0..3, n cols 4..5
_BLKRZ = {(0, 0): 0, (0, 1): 1, (1, 0): 2, (1, 1): 3}


def _sub(base, off, dims):
    """Manual AP: base is a [128, X] AP; append free dims after partition."""
    return bass.AP(tensor=base.tensor, offset=base.offset + off,
                   ap=[base.ap[0]] + dims)


def _v(t, dims, off=0):
    return bass.AP(tensor=t.tensor, offset=t.offset + off,
                   ap=[t.ap[0]] + dims)


def _split_waits(nc, cap=1):
    """Hoist excess sync waits onto standalone event-semaphore ops.

    The walrus build on this stack accepts only `cap` sync-wait commands
    per ISA instruction; Tile can attach several. Event-semaphore ops on
    the same engine execute in queue order, so hoisting preserves
    semantics.
    """
    n = 0
    for fn in nc.m.functions:
        for bb in fn.blocks:
            out = []
            for ins in bb.instructions:
                si = ins.sync_info
                if si is not None and len(si.on_wait) > cap:
                    waits = list(si.on_wait)
                    keep = waits[len(waits) - cap:] if cap else []
                    for w in waits[:len(waits) - cap] if cap else waits:
                        n += 1
                        out.append(mybir.InstEventSemaphore(
                            name=f"wsplit-{n}", engine=ins.engine,
                            ins=[], outs=[],
                            sync_info=mybir.SyncInfo(on_wait=[w],
                                                     on_update=[])))
                    ins.sync_info = mybir.SyncInfo(
                        on_wait=keep, on_update=list(si.on_update))
                out.append(ins)
            bb.instructions = out
    return n


def _build(zero_bhn: bool, debug: bool = False):
    nc = bass.Bass()
    if debug:
        dbg_hs_d = nc.declare_dram_parameter("dbg_hs", [128, 2 * (S + 1) * BL],
                                             F16, True)
        dbg_uh_d = nc.declare_dram_parameter("dbg_uh", [128, NGRP * 160], F16,
                                             True)
        dbg_o_d = nc.declare_dram_parameter("dbg_o", [BL, 160], F32, True)

    xidx_d = nc.declare_dram_parameter("xidx", [128, NGRP], I32, False)
    emb_d = nc.declare_dram_parameter("emb", [VOCAB, DWP], F16, False)
    wih_d = nc.declare_dram_parameter("wih", [2, 3, 128, G3], F16, False)
    whh_d = nc.declare_dram_parameter("whh", [2, H, G3], F16, False)
    biasx_d = nc.declare_dram_parameter("biasx", [128, 6], F32, False)
    bhn_d = nc.declare_dram_parameter("bhn", [128, 2], F32, False)
    wcap_d = nc.declare_dram_parameter("wcap", [2, H, 160], F16, False)
    wlin_d = nc.declare_dram_parameter("wlin", [160, 2], F16, False)
    blin_d = nc.declare_dram_parameter("blin", [2, 1], F32, False)
    selB_d = nc.declare_dram_parameter("selB", [128, BL], F16, False)
    selT_d = nc.declare_dram_parameter("selT", [BL, 128], F16, False)
    ident_d = nc.declare_dram_parameter("ident", [128, 128], F16, False)
    out_d = nc.declare_dram_parameter("out", [BL, 2], F32, True)

    with tile.TileContext(nc) as tc, ExitStack() as ctx:
        const = ctx.enter_context(tc.tile_pool(name="const", bufs=1))
        bigxp = ctx.enter_context(tc.tile_pool(name="bigxp", bufs=1))
        bighs = ctx.enter_context(tc.tile_pool(name="bighs", bufs=1))
        work = ctx.enter_context(tc.tile_pool(name="work", bufs=3))

        # ---- constants to SBUF ----
        xidx = const.tile([128, NGRP], I32)
        nc.sync.dma_start(out=xidx[:], in_=xidx_d[:, :])
        whh = const.tile([128, 2, G3], F16)
        for d in range(2):
            nc.sync.dma_start(out=whh[:, d, :], in_=whh_d[d, :, :])
        biasx = const.tile([128, 6], F32)
        nc.sync.dma_start(out=biasx[:], in_=biasx_d[:, :])
        bhn = const.tile([128, 2], F32)
        nc.sync.dma_start(out=bhn[:], in_=bhn_d[:, :])
        wcap = const.tile([128, 2, 160], F16)
        for k in range(2):
            nc.sync.dma_start(out=wcap[:, k, :], in_=wcap_d[k, :, :])
        wlin = const.tile([128, 2, 2], F16)        # chunk0 [:128], chunk1 [:32]
        nc.sync.dma_start(out=wlin[:, 0, :], in_=wlin_d[0:128, :])
        nc.sync.dma_start(out=wlin[:32, 1, :], in_=wlin_d[128:160, :])
        blin = const.tile([2, 1], F32)
        nc.sync.dma_start(out=blin[:], in_=blin_d[:, :])
        selB = const.tile([128, BL], F16)
        nc.sync.dma_start(out=selB[:], in_=selB_d[:, :])
        selT = const.tile([BL, 128], F16)
        nc.sync.dma_start(out=selT[:], in_=selT_d[:, :])
        identb = const.tile([128, 128], F16)
        nc.sync.dma_start(out=identb[:], in_=ident_d[:, :])
        epst = const.tile([128, 1], F32)
        nc.vector.memset(epst[:], EPS)
        mhalf = const.tile([128, 1], F32)
        nc.vector.memset(mhalf[:], -0.5)
        mone = const.tile([128, 1], F32)
        nc.vector.memset(mone[:], -1.0)

        xprz = bigxp.tile([128, EXT * RZW], F16)
        xpn = bigxp.tile([128, EXT * NW], F16)
        hbf = bighs.tile([128, 2 * (S + 1) * BL], F16)   # token-major
        hbf_s = bighs.tile([128, (2 * CCH + 1) * PB], F16)  # slot-major
        # warmup scratch: per dir [zero, ping, pong] x PB
        hsc = bighs.tile([128, 2, 3, PB], F16)
        nc.gpsimd.memset(hsc[:], 0.0)
        # warmup pads force h -> 0 exactly: r=sigmoid(-30)=0, w=sigmoid(30)=1,
        # xn=0  =>  h' = 1*tanh(0) + 0*h = 0
        for p0 in (0, S + WU):
            for blk, val in ((0, -30.0), (1, 30.0), (2, -30.0), (3, 30.0)):
                nc.vector.memset(_sub(xprz[:], p0 * RZW + blk * BL,
                                      [[RZW, WU], [1, BL]]), val)
            nc.gpsimd.memset(_sub(xpn[:], p0 * NW, [[1, WU * NW]]), 0.0)

        # ---- phases B+C: gather + DMA-transpose + x_proj ----
        with tc.tile_pool(name="bc", bufs=1) as bc, \
             tc.tile_pool(name="ps_bc", bufs=1, space="PSUM") as ps_bc:
            wih = bc.tile([128, 2, 3, G3], F16)    # [dimpart, dir, kchunk, gcol]
            for d in range(2):
                for b in range(3):
                    nc.sync.dma_start(out=wih[:, d, b, :],
                                      in_=wih_d[d, b, :, :])
            g = bc.tile([128, NGRP * DWP], F16)
            eT = bc.tile([128, 3, NTOK], F16)
            # chunk-major pipeline: gather+transpose the 4 groups a chunk
            # needs, then immediately run that chunk's 6 x_proj matmul
            # triples so the PE chases the gather stream
            for ch in range(8):
                for i in range(4 * ch, 4 * ch + 4):
                    nc.gpsimd.indirect_dma_start(
                        out=g[:, i * DWP:(i + 1) * DWP], out_offset=None,
                        in_=emb_d[:, :],
                        in_offset=IndirectOffsetOnAxis(ap=xidx[:, i:i + 1],
                                                       axis=0))
                    for b in range(3):
                        pt = ps_bc.tile([128, 128], F32, tag="pt", bufs=4)
                        nc.tensor.matmul(
                            pt[:],
                            lhsT=g[:, i * DWP + b * 128:i * DWP + b * 128 + 128],
                            rhs=identb[:], start=True, stop=True)
                        if (i + b) % 2 == 0:
                            nc.vector.tensor_copy(
                                eT[:, b, i * 128:(i + 1) * 128], pt[:])
                        else:
                            nc.scalar.copy(
                                eT[:, b, i * 128:(i + 1) * 128], pt[:])
                for d in range(2):
                    for gt in range(3):
                        px = ps_bc.tile([128, 512], F32, tag="px", bufs=3)
                        for b in range(3):
                            nc.tensor.matmul(
                                px[:, :],
                                lhsT=wih[:, d, b, gt * H:(gt + 1) * H],
                                rhs=eT[:, b, ch * 512:(ch + 1) * 512],
                                start=(b == 0), stop=(b == 2))
                        src = _v(px, [[BL, 32], [1, BL]])
                        if gt < 2:
                            blk = _BLKRZ[(d, gt)]
                            dst = _sub(xprz[:], (WU + ch * 32) * RZW + blk * BL,
                                       [[RZW, 32], [1, BL]])
                            bcol = blk
                        else:
                            dst = _sub(xpn[:], (WU + ch * 32) * NW + d * BL,
                                       [[NW, 32], [1, BL]])
                            bcol = 4 + d
                        if (d * 3 + gt + ch) % 2 == 0:
                            nc.vector.tensor_scalar_add(
                                dst, src, biasx[:, bcol:bcol + 1])
                        else:
                            nc.scalar.activation(
                                dst, src, AF.Identity,
                                bias=biasx[:, bcol:bcol + 1])

        # ---- phase D: chunked-parallel scan ----
        def h_read(k, d):
            if k == 0:
                return _sub(hsc[:], (d * 3 + 0) * PB, [[1, PB]])
            if k <= WU:
                return _sub(hsc[:], (d * 3 + 1 + ((k - 1) % 2)) * PB,
                            [[1, PB]])
            if d == 0:
                off = (k - WU) * PB
            else:
                off = HB2 + (CCH + WU - k) * PB
            return _sub(hbf_s[:], off, [[1, PB]])

        def h_write(k, d):
            if k < WU:
                return _sub(hsc[:], (d * 3 + 1 + (k % 2)) * PB, [[1, PB]])
            if d == 0:
                off = (k - WU + 1) * PB
            else:
                off = HB2 + (CCH - 1 + WU - k) * PB
            return _sub(hbf_s[:], off, [[1, PB]])

        with tc.tile_pool(name="ps_scan", bufs=1, space="PSUM") as ps_sc:
            for k in range(WU + CCH):
                for d in range(2):
                    prz = ps_sc.tile([128, 2 * PB], F32, tag=f"prz{d}",
                                     bufs=2)
                    pn = ps_sc.tile([128, PB], F32, tag=f"pn{d}", bufs=2)
                    xo = k if d == 0 else (CCH - 1 + 2 * WU - k)
                    h_rd = h_read(k, d)
                    xr_r = _sub(xprz[:], xo * RZW + d * 2 * BL,
                                [[CCH * RZW, PCH], [1, BL]])
                    xr_z = _sub(xprz[:], xo * RZW + d * 2 * BL + BL,
                                [[CCH * RZW, PCH], [1, BL]])
                    mi_r = nc.tensor.matmul(prz[:, 0:PB], lhsT=identb[:],
                                            rhs=xr_r, start=True, stop=False)
                    g_r = nc.tensor.matmul(prz[:, 0:PB],
                                           lhsT=whh[:, d, 0:H], rhs=h_rd,
                                           start=False, stop=True)
                    add_dep_helper(g_r.ins, mi_r.ins, sync=False, reason="acc")
                    mi_z = nc.tensor.matmul(prz[:, PB:2 * PB], lhsT=identb[:],
                                            rhs=xr_z, start=True, stop=False)
                    g_z = nc.tensor.matmul(prz[:, PB:2 * PB],
                                           lhsT=whh[:, d, H:2 * H], rhs=h_rd,
                                           start=False, stop=True)
                    add_dep_helper(g_z.ins, mi_z.ins, sync=False, reason="acc")
                    nc.tensor.matmul(pn[:], lhsT=whh[:, d, 2 * H:3 * H],
                                     rhs=h_rd, start=True, stop=True)

                    rw = work.tile([128, 2 * PB], F16, tag=f"rw{d}")
                    nc.scalar.activation(rw[:, 0:PB], prz[:, 0:PB],
                                         AF.Sigmoid)
                    nc.scalar.activation(rw[:, PB:2 * PB], prz[:, PB:2 * PB],
                                         AF.Sigmoid)
                    vt = work.tile([128, PB], F16, tag=f"v{d}")
                    nc.gpsimd.tensor_scalar(vt[:], rw[:, PB:2 * PB],
                                            -1.0, 1.0,
                                            op0=OP.mult, op1=OP.add)
                    tn = work.tile([128, PB], F16, tag=f"tn{d}")
                    if zero_bhn:
                        nc.vector.tensor_tensor(tn[:], pn[:], rw[:, 0:PB],
                                                op=OP.mult)
                    else:
                        nc.vector.scalar_tensor_tensor(
                            tn[:], pn[:], bhn[:, d:d + 1], rw[:, 0:PB],
                            op0=OP.add, op1=OP.mult)
                    t2 = work.tile([128, PB], F16, tag=f"t2{d}")
                    xn_ap = _sub(xpn[:], xo * NW + d * BL,
                                 [[CCH * NW, PCH], [1, BL]])
                    nc.vector.tensor_tensor(_v(t2, [[BL, PCH], [1, BL]]),
                                            _v(tn, [[BL, PCH], [1, BL]]),
                                            xn_ap, op=OP.add)
                    n_t = work.tile([128, PB], F16, tag=f"n{d}")
                    nc.scalar.activation(n_t[:], t2[:], AF.Tanh)

                    # h' = w*n + (1-w)*h
                    u_t = work.tile([128, PB], F16, tag=f"u{d}")
                    nc.vector.tensor_tensor(u_t[:], rw[:, PB:2 * PB], n_t[:],
                                            op=OP.mult)
                    p2 = work.tile([128, PB], F16, tag=f"p2{d}")
                    nc.gpsimd.tensor_tensor(p2[:], vt[:], h_rd, op=OP.mult)
                    nc.vector.tensor_tensor(h_write(k, d), u_t[:], p2[:],
                                            op=OP.add)
                    if k >= WU:
                        if d == 0:
                            j = k - WU + 1
                            s_off, t_off = j * PB, j * BL
                        else:
                            j = CCH - 1 + WU - k
                            s_off, t_off = HB2 + j * PB, HB0 + j * BL
                        nc.sync.dma_start(
                            out=_sub(hbf[:], t_off,
                                     [[CCH * BL, PCH], [1, BL]]),
                            in_=_sub(hbf_s[:], s_off, [[1, PB]]))

        if debug:
            nc.sync.dma_start(out=dbg_hs_d[:, :], in_=hbf[:])

        # ---- phases E/F/G ----
        with tc.tile_pool(name="ef", bufs=1) as ef, \
             tc.tile_pool(name="rp", bufs=1) as rp, \
             tc.tile_pool(name="ps_ef", bufs=1, space="PSUM") as ps_ef:
            # capsule u_hat [tok, 160] fp16, dc-major columns
            uh = ef.tile([128, NGRP * 160], F16)
            for c in range(NGRP):
                pu = ps_ef.tile([128, 160], F32, tag="pu", bufs=2)
                lhs_f = _sub(hbf[:], (1 + 8 * c) * BL, [[1, 128]])
                lhs_b = _sub(hbf[:], HB0 + 8 * c * BL, [[1, 128]])
                m1 = nc.tensor.matmul(pu[:], lhsT=lhs_f, rhs=wcap[:, 0, :],
                                      start=True, stop=False)
                m2 = nc.tensor.matmul(pu[:], lhsT=lhs_b, rhs=wcap[:, 1, :],
                                      start=False, stop=True)
                add_dep_helper(m2.ins, m1.ins, sync=False, reason="acc")
                if c % 2 == 0:
                    nc.vector.tensor_copy(uh[:, c * 160:(c + 1) * 160], pu[:])
                else:
                    nc.scalar.copy(uh[:, c * 160:(c + 1) * 160], pu[:])

            if debug:
                nc.sync.dma_start(out=dbg_uh_d[:, :], in_=uh[:])

            # routing (u_hat/tmp dc-major: col = dc*10 + j; bl/c j-innermost)
            c_t = rp.tile([128, NGRP * NUM_CAP], F16, tag="c")
            nc.vector.memset(c_t[:], 1.0 / NUM_CAP)
            bl_t = rp.tile([128, NGRP * NUM_CAP], F32, tag="bl")
            nc.gpsimd.memset(bl_t[:], 0.0)
            outputs = rp.tile([BL, 160], F16, tag="outs")
            tmp = rp.tile([128, NGRP * 160], F16, tag="tmp")
            du1 = rp.tile([128, NGRP * 80], F16, tag="du1", bufs=2)
            du2 = rp.tile([128, NGRP * 40], F16, tag="du2", bufs=2)
            du3 = rp.tile([128, NGRP * 20], F16, tag="du3", bufs=2)
            du4 = rp.tile([128, NGRP * 10], F16, tag="du4", bufs=2)

            for it in range(ROUTINGS):
                if it > 0:
                    # softmax over capsules (groups of 10); b is bounded so
                    # no max-subtraction needed, but exp stays f32
                    sbe = rp.tile([128, NGRP * NUM_CAP], F32, tag="sbe",
                                  bufs=2)
                    nc.scalar.activation(sbe[:], bl_t[:], AF.Exp)
                    sm = rp.tile([128, NGRP], F32, tag="sm", bufs=2)
                    nc.vector.tensor_reduce(
                        sm[:], _v(sbe, [[NUM_CAP, NGRP], [1, NUM_CAP]]),
                        axis=AX.X, op=OP.add)
                    rc = rp.tile([128, NGRP], F32, tag="rc", bufs=2)
                    nc.vector.reciprocal(rc[:], sm[:])
                    nc.vector.tensor_tensor(
                        _v(c_t, [[NUM_CAP, NGRP], [1, NUM_CAP]]),
                        _v(sbe, [[NUM_CAP, NGRP], [1, NUM_CAP]]),
                        _v(rc, [[1, NGRP], [0, NUM_CAP]]), op=OP.mult)

                # tmp = u_hat * c (c broadcast over dc), sum over s via
                # fp16 selB matmuls, pipelined in 4 chunks of 8 groups
                po = ps_ef.tile([BL, 160], F32, tag="po", bufs=2)
                prev = None
                for q in range(4):
                    lo = q * 8
                    nc.vector.tensor_tensor(
                        _sub(tmp[:], lo * 160,
                             [[160, 8], [NUM_CAP, DIM_CAP], [1, NUM_CAP]]),
                        _sub(uh[:], lo * 160,
                             [[160, 8], [NUM_CAP, DIM_CAP], [1, NUM_CAP]]),
                        _sub(c_t[:], lo * NUM_CAP,
                             [[NUM_CAP, 8], [0, DIM_CAP], [1, NUM_CAP]]),
                        op=OP.mult)
                    for j in range(lo, lo + 8):
                        mm = nc.tensor.matmul(
                            po[:], lhsT=selB[:],
                            rhs=tmp[:, j * 160:(j + 1) * 160],
                            start=(j == 0), stop=(j == NGRP - 1))
                        if prev is not None:
                            add_dep_helper(mm.ins, prev.ins, sync=False,
                                           reason="acc")
                        prev = mm
                # squash: out = po / sqrt(sum(po^2) + eps); 1/sqrt via
                # exp(-0.5*ln(s+eps)) to stay within one ACT table set
                po_s = rp.tile([BL, 160], F16, tag="po_s", bufs=2)
                nc.vector.tensor_copy(po_s[:], po[:])
                sq = rp.tile([BL, 160], F16, tag="sq", bufs=2)
                nc.vector.tensor_tensor(sq[:], po_s[:], po_s[:], op=OP.mult)
                ssum = rp.tile([BL, NUM_CAP], F32, tag="ssum", bufs=2)
                nc.vector.tensor_reduce(
                    ssum[:], _v(sq, [[1, NUM_CAP], [NUM_CAP, DIM_CAP]]),
                    axis=AX.X, op=OP.add)
                lns = rp.tile([BL, NUM_CAP], F32, tag="lns", bufs=2)
                nc.scalar.activation(lns[:], ssum[:], AF.Ln,
                                     bias=epst[:BL, 0:1])
                rs = rp.tile([BL, NUM_CAP], F32, tag="rs", bufs=2)
                nc.scalar.activation(rs[:], lns[:], AF.Exp,
                                     scale=mhalf[:BL, 0:1])
                nc.vector.tensor_tensor(
                    outputs[:], po_s[:],
                    _v(rs, [[0, DIM_CAP], [1, NUM_CAP]]), op=OP.mult)

                if it < ROUTINGS - 1:
                    # broadcast outputs to all 128 partitions via selT matmul
                    pob = ps_ef.tile([128, 160], F32, tag="pob", bufs=1)
                    nc.tensor.matmul(pob[:], lhsT=selT[:], rhs=outputs[:],
                                     start=True, stop=True)
                    obb = rp.tile([128, 160], F16, tag="obb", bufs=2)
                    nc.vector.tensor_copy(obb[:], pob[:])
                    # du = sum_dc u_hat * ob: fp16 2x multiply + add-tree
                    nc.vector.tensor_tensor(
                        _v(tmp, [[160, NGRP], [1, 160]]),
                        _v(uh, [[160, NGRP], [1, 160]]),
                        _v(obb, [[0, NGRP], [1, 160]]), op=OP.mult)
                    nc.vector.tensor_tensor(
                        _v(du1, [[80, NGRP], [1, 80]]),
                        _v(tmp, [[160, NGRP], [1, 80]]),
                        _v(tmp, [[160, NGRP], [1, 80]], off=80), op=OP.add)
                    nc.vector.tensor_tensor(
                        _v(du2, [[40, NGRP], [1, 40]]),
                        _v(du1, [[80, NGRP], [1, 40]]),
                        _v(du1, [[80, NGRP], [1, 40]], off=40), op=OP.add)
                    nc.vector.tensor_tensor(
                        _v(du3, [[20, NGRP], [1, 20]]),
                        _v(du2, [[40, NGRP], [1, 20]]),
                        _v(du2, [[40, NGRP], [1, 20]], off=20), op=OP.add)
                    nc.vector.tensor_tensor(
                        _v(du4, [[10, NGRP], [1, 10]]),
                        _v(du3, [[20, NGRP], [1, 10]]),
                        _v(du3, [[20, NGRP], [1, 10]], off=10), op=OP.add)
                    nc.vector.tensor_tensor(bl_t[:], bl_t[:], du4[:],
                                            op=OP.add)

            if debug:
                nc.sync.dma_start(out=dbg_o_d[:, :], in_=outputs[:])

            # final linear (dc-major wlin rows match outputs layout)
            pt1 = ps_ef.tile([128, BL], F32, tag="pt1", bufs=1)
            nc.tensor.matmul(pt1[:, :], lhsT=outputs[:, 0:128],
                             rhs=identb[:BL, :BL], start=True, stop=True)
            pt2 = ps_ef.tile([32, BL], F32, tag="pt2", bufs=1)
            nc.tensor.matmul(pt2[:, :], lhsT=outputs[:, 128:160],
                             rhs=identb[:BL, :BL], start=True, stop=True)
            capsT = rp.tile([128, 2 * BL], F16, tag="capsT")
            nc.vector.tensor_copy(capsT[:, 0:BL], pt1[:])
            nc.vector.tensor_copy(capsT[:32, BL:2 * BL], pt2[:])
            pf = ps_ef.tile([2, BL], F32, tag="pf", bufs=1)
            mf1 = nc.tensor.matmul(pf[:], lhsT=wlin[:, 0, :],
                                   rhs=capsT[:, 0:BL], start=True, stop=False)
            mf2 = nc.tensor.matmul(pf[:], lhsT=wlin[:32, 1, :],
                                   rhs=capsT[:32, BL:2 * BL],
                                   start=False, stop=True)
            add_dep_helper(mf2.ins, mf1.ins, sync=False, reason="acc")
            outT = rp.tile([2, BL], F32, tag="outT")
            nc.scalar.activation(outT[:], pf[:], AF.Identity,
                                 bias=blin[:, 0:1])
            dst = bass.AP(tensor=out_d, offset=0, ap=[[1, 2], [2, BL]])
            nc.sync.dma_start(out=dst, in_=outT[:])

    return nc


_CACHE = {}


def _get_nc(zero_bhn):
    if zero_bhn not in _CACHE:
        nc = _build(zero_bhn)
        _split_waits(nc)   # HW-path legalization (CoreSim path builds its own)
        _CACHE[zero_bhn] = nc
    return _CACHE[zero_bhn]


_HOST_CACHE = {}


def _host_inputs(x, emb, w_ih_f, w_hh_f, b_ih_f, b_hh_f,
                 w_ih_b, w_hh_b, b_ih_b, b_hh_b, W_cap, W_lin, b_lin):
    """Build the per-core input maps (everything but xidx is shared)."""
    f32 = np.float32
    f16 = np.float16
    neg = np.ones((G3,), f32)
    neg[H:2 * H] = -1.0        # negate z gate (sigmoid -> 1-z)

    # emb padded to 384 dims, fp16
    embp = np.zeros((VOCAB, DWP), f16)
    embp[:, :D_W] = np.asarray(emb, f32).astype(f16)

    # wih: [dir, kchunk, dimpart, gatecol]; dim index = kchunk*128 + dimpart
    wih = np.zeros((2, 3, 128, G3), f16)
    for d, w in enumerate([w_ih_f, w_ih_b]):
        wt = (np.asarray(w, f32).T * neg)          # [D_W, G3]
        wtp = np.zeros((DWP, G3), f32)
        wtp[:D_W] = wt
        wih[d] = wtp.reshape(3, 128, G3).astype(f16)
    whh = np.stack([(np.asarray(w_hh_f, f32).T * neg).astype(f16),
                    (np.asarray(w_hh_b, f32).T * neg).astype(f16)])

    biasx = np.zeros((128, 6), f32)
    for d, (bi, bh) in enumerate([(b_ih_f, b_hh_f), (b_ih_b, b_hh_b)]):
        biasx[:, _BLKRZ[(d, 0)]] = (bi[0:H] + bh[0:H])
        biasx[:, _BLKRZ[(d, 1)]] = -(bi[H:2 * H] + bh[H:2 * H])
        biasx[:, 4 + d] = bi[2 * H:3 * H]
    bhn = np.zeros((128, 2), f32)
    bhn[:, 0] = b_hh_f[2 * H:3 * H]
    bhn[:, 1] = b_hh_b[2 * H:3 * H]
    zero_bhn = bool(np.all(bhn == 0.0))

    # dc-major permutation: new col dc*10+j  <- old col j*16+dc
    perm = np.empty((160,), np.int64)
    for j in range(NUM_CAP):
        for dc in range(DIM_CAP):
            perm[dc * NUM_CAP + j] = j * DIM_CAP + dc
    Wc = np.asarray(W_cap, f32)[:, perm]
    wcap = np.stack([Wc[0:H, :].astype(f16), Wc[H:2 * H, :].astype(f16)])
    wlin = np.ascontiguousarray(np.asarray(W_lin, f32)[perm, :]).astype(f16)

    selB = (np.arange(128)[:, None] % BL == np.arange(BL)[None, :]).astype(f16)
    selT = selB.T.copy()
    ident = np.eye(128, dtype=f16)

    shared = dict(emb=embp, wih=wih, whh=whh,
                  biasx=biasx, bhn=bhn, wcap=wcap, wlin=wlin,
                  blin=np.ascontiguousarray(b_lin, f32).reshape(2, 1),
                  selB=selB, selT=selT, ident=ident)

    in_maps = []
    for c in range(NCORES):
        xl = np.asarray(x[c * BL:(c + 1) * BL, :])          # [BL, S]
        tok = xl.T.reshape(-1).astype(np.int32)             # s-major [NTOK]
        xidx = np.ascontiguousarray(tok.reshape(NGRP, 128).T)  # [128, NGRP]
        in_maps.append(dict(shared, xidx=xidx))
    return in_maps, zero_bhn


def kernel(**inputs):
    in_maps, zero_bhn = _host_inputs(**{k: np.asarray(v) for k, v in
                                        inputs.items()})
    nc = _get_nc(zero_bhn)
    res = run_bass_kernel_spmd(nc, in_maps, list(range(NCORES)))
    return np.concatenate([res.results[c]["out"] for c in range(NCORES)],
                          axis=0)


def _install_ntff_hook():
    """Shim the missing antenv.axon_hooks so trace=True works under axon."""
    import sys, types
    if "antenv.axon_hooks" in sys.modules:
        return
    mod = types.ModuleType("antenv.axon_hooks")
    _h = [None]
    mod.set_axon_ntff_profile_hook = lambda h: _h.__setitem__(0, h)
    mod.get_axon_ntff_profile_hook = lambda: _h[0]
    sys.modules["antenv.axon_hooks"] = mod
    import antenv
    antenv.axon_hooks = mod
    from trn_agent_boot.trn_boot import _ntff_profile_via_ctypes
    mod.set_axon_ntff_profile_hook(
        _ntff_profile_via_ctypes("/opt/axon/libaxon_pjrt.so"))


def kernel_profiled(**inputs):
    """Same as kernel() but with NTFF tracing; returns (out, result_obj)."""
    _install_ntff_hook()
    in_maps, zero_bhn = _host_inputs(**{k: np.asarray(v) for k, v in
                                        inputs.items()})
    nc = _get_nc(zero_bhn)
    res = run_bass_kernel_spmd(nc, in_maps, list(range(NCORES)), trace=True)
    out = np.concatenate([res.results[c]["out"] for c in range(NCORES)],
                         axis=0)
    return out, res
